# revision 1
# baseline (speedup 1.0000x reference)
"""Vocab-parallel softmax(x @ A.T) on 8 TRN2 NeuronCores.

Problem: input x [32, 1024] f32, atom_matrix A [128000, 1024] f32.
Output: softmax(x @ A.T, axis=-1) [32, 128000] f32.

Strategy (memory-bound: A is 512 MB):
  - Shard A row-wise (vocab dim) -> 16000 atoms/core.
  - Host pre-permutes each shard into chunk-blocked transposed layout
    [c*128+p, k*CHUNK+a] = A^T[k*128+p, c*CHUNK+a] and casts to fp16, so
    each 500-atom chunk is ONE fully-contiguous 1 MB DMA with the d
    (contraction) axis on SBUF partitions. fp16 halves HBM traffic vs
    f32 while keeping softmax error ~5e-4 absmax-rel (1.5e-3 worst
    elementwise) -- far inside the 2e-2 gate.
  - Per core: stream chunks, matmul (x^T stationary, fp32 PSUM
    accumulate) into PSUM [32, 500] logits, Exp via ScalarE activation
    with accum_out giving per-chunk partial sums, all in one pass.
  - AllGather the per-core [32] exp-sums (256 B), reduce locally,
    normalize by 1/S, DMA out. Logits are O(1) by construction
    (LOGIT_SCALE in the model), so max-subtraction is unnecessary:
    |logit| <~ 5, exp(logit) <= ~150, sums ~1e5 -- all fine in fp32.
"""

import numpy as np

BATCH = 32
D = 1024
N_ATOMS = 128000
N_CORES = 8
SHARD = N_ATOMS // N_CORES  # 16000
KT = D // 128               # 8 contraction tiles
CHUNK = 500                 # atoms per PSUM tile (walrus caps moving dim at 512)
NCH = SHARD // CHUNK        # 32 chunks

# Compute dtype for A / x. fp16 halves HBM traffic for A (the dominant
# cost); 10 mantissa bits keep softmax err ~5e-4. PSUM accumulates fp32.
DTYPE = "fp16"

_state = {}


def _mybir_dt(dtype_name):
    import concourse.mybir as mybir
    return {"f32": mybir.dt.float32,
            "bf16": mybir.dt.bfloat16,
            "fp16": mybir.dt.float16}[dtype_name]


def _np_cdt(dtype_name=None):
    dtype_name = dtype_name or DTYPE
    if dtype_name == "f32":
        return np.float32
    if dtype_name == "fp16":
        return np.float16
    import ml_dtypes
    return ml_dtypes.bfloat16


def _build(repeat=1, dtype_name=None, probe=False, super_chunks=4,
           exp16=False, dma_split=False, ps_bufs=2, a_bufs=None):
    """probe=True: A becomes uninitialized Internal DRAM (same bytes
    streamed; tiny inputs) and Exp runs with scale=0 so garbage contents
    never produce NaN notifications. Used only for exec-time measurement.

    super_chunks: how many 500-atom chunks ride in one DMA (bigger DMAs,
    8 KB contiguous runs per partition)."""
    import concourse.mybir as mybir
    import concourse.tile as tile
    from concourse import bacc

    dtype_name = dtype_name or DTYPE
    f32 = mybir.dt.float32
    cdt = _mybir_dt(dtype_name)

    nc = bacc.Bacc("TRN2", target_bir_lowering=False, debug=False,
                   num_devices=N_CORES)
    xT = nc.dram_tensor("xT", [D, BATCH], cdt, kind="ExternalInput").ap()
    # chunk-blocked A^T: row c*128+p, col k*CHUNK+a  (see make_in_maps)
    at = nc.dram_tensor("at", [NCH * 128, KT * CHUNK], cdt,
                        kind="Internal" if probe else "ExternalInput").ap()
    out = nc.dram_tensor("out", [BATCH, SHARD], f32, kind="ExternalOutput").ap()

    with tile.TileContext(nc) as tc:
        if a_bufs is None:
            a_bufs = {1: 4, 2: 4, 4: 3}.get(super_chunks, 2)
            if exp16 and super_chunks == 4:
                a_bufs = 4  # fp16 exp_buf frees 32 KB/partition
        o_bufs = 2 if super_chunks >= 8 or a_bufs >= 5 else 4
        exp_dt = _mybir_dt("fp16") if exp16 else f32
        with (
            tc.tile_pool(name="xp", bufs=1) as xpool,
            tc.tile_pool(name="apool", bufs=a_bufs) as apool,
            tc.tile_pool(name="pp", bufs=ps_bufs, space="PSUM") as pspool,
            tc.tile_pool(name="bigp", bufs=1) as bigpool,
            tc.tile_pool(name="smallp", bufs=1) as smallpool,
            tc.tile_pool(name="outp", bufs=o_bufs) as outpool,
            tc.tile_pool(name="dramp", bufs=1, space="DRAM") as drampool,
        ):
            for rep in range(repeat):
                if rep:
                    tc.strict_bb_all_engine_barrier()
                # x^T tiled by contraction: SBUF [128, KT, 32]; k-tile k
                # holds x^T rows k*128..(k+1)*128 (partition p <-> k*128+p).
                xs = xpool.tile([128, KT, BATCH], cdt, name="xs")
                nc.sync.dma_start(xs, xT.rearrange("(k p) b -> p k b", p=128))

                exp_buf = bigpool.tile([BATCH, SHARD], exp_dt, name="exp_buf")
                sums = smallpool.tile([BATCH, NCH], f32, name="sums")

                SC = super_chunks
                assert NCH % SC == 0
                for sc in range(NCH // SC):
                    a_t = apool.tile([128, SC, KT * CHUNK], cdt, name="a_t")
                    if dma_split and SC % 2 == 0:
                        half = SC // 2
                        lo = at[sc * SC * 128:(sc * SC + half) * 128, :]
                        hi = at[(sc * SC + half) * 128:(sc + 1) * SC * 128, :]
                        nc.sync.dma_start(
                            a_t[:, 0:half, :],
                            lo.rearrange("(s p) f -> p s f", p=128))
                        nc.scalar.dma_start(
                            a_t[:, half:SC, :],
                            hi.rearrange("(s p) f -> p s f", p=128))
                    else:
                        src = at[sc * SC * 128:(sc + 1) * SC * 128, :].rearrange(
                            "(s p) f -> p s f", p=128)
                        nc.sync.dma_start(a_t, src)
                    for t in range(SC):
                        c = sc * SC + t
                        ps = pspool.tile([BATCH, CHUNK], f32, name="ps")
                        for k in range(KT):
                            nc.tensor.matmul(
                                ps, lhsT=xs[:, k, :],
                                rhs=a_t[:, t, k * CHUNK:(k + 1) * CHUNK],
                                start=(k == 0), stop=(k == KT - 1))
                        # exp(logits) -> SBUF, plus per-partition partial sums
                        nc.scalar.activation(
                            exp_buf[:, c * CHUNK:(c + 1) * CHUNK], ps,
                            mybir.ActivationFunctionType.Exp,
                            scale=0.0 if probe else 1.0,
                            accum_out=sums[:, c:c + 1])

                # Local sum over chunks -> [32, 1]
                lsum = smallpool.tile([BATCH, 1], f32, name="lsum")
                nc.vector.reduce_sum(lsum, sums, axis=mybir.AxisListType.X)

                # AllGather per-core sums (256 B), reduce locally.
                cc_in = drampool.tile([BATCH, 1], f32, name="cc_in")
                cc_out = drampool.tile([N_CORES, BATCH], f32,
                                       addr_space="Shared", name="cc_out")
                nc.sync.dma_start(cc_in, lsum)
                nc.gpsimd.collective_compute(
                    "AllGather", mybir.AluOpType.bypass,
                    replica_groups=[list(range(N_CORES))],
                    ins=[cc_in.opt()], outs=[cc_out.opt()])
                gat = smallpool.tile([BATCH, N_CORES], f32, name="gat")
                # transpose-on-read: partition b <- gathered[:, b]
                nc.sync.dma_start(gat, cc_out.rearrange("r b -> b r"))
                gsum = smallpool.tile([BATCH, 1], f32, name="gsum")
                nc.vector.reduce_sum(gsum, gat, axis=mybir.AxisListType.X)
                rinv = smallpool.tile([BATCH, 1], f32, name="rinv")
                nc.vector.reciprocal(rinv, gsum)

                # Normalize and store, sliced for DMA overlap; alternate
                # ScalarE / VectorE so both engines share the tail.
                NS = 8
                W = SHARD // NS
                for s in range(NS):
                    sl = slice(s * W, (s + 1) * W)
                    ot = outpool.tile([BATCH, W], f32, name="ot")
                    if s % 2 == 0:
                        nc.scalar.mul(ot, exp_buf[:, sl], rinv)
                    else:
                        nc.vector.tensor_scalar_mul(ot, exp_buf[:, sl], rinv)
                    nc.sync.dma_start(out[:, sl], ot)

    nc.compile()
    return nc


def _get_nc():
    if "nc" not in _state:
        _state["nc"] = _build()
    return _state["nc"]


def make_in_maps(input, atom_matrix, dtype_name=None):
    dtype_name = dtype_name or DTYPE
    cdt = _np_cdt(dtype_name)
    xT = np.ascontiguousarray(input.T).astype(cdt)
    in_maps = []
    for i in range(N_CORES):
        shard = atom_matrix[i * SHARD:(i + 1) * SHARD, :]  # [16000, 1024]
        att = shard.T.astype(cdt)                          # [1024, 16000]
        # blocked[c*128+p, k*CHUNK+a] = A^T[k*128+p, c*CHUNK+a]
        at_i = np.ascontiguousarray(
            att.reshape(KT, 128, NCH, CHUNK)
               .transpose(2, 1, 0, 3)
               .reshape(NCH * 128, KT * CHUNK))
        in_maps.append({"xT": xT, "at": at_i})
    return in_maps


def kernel(input, atom_matrix):
    from concourse import bass_utils

    input = np.asarray(input)
    atom_matrix = np.asarray(atom_matrix)
    nc = _get_nc()
    in_maps = make_in_maps(input, atom_matrix)
    res = bass_utils.run_bass_kernel_spmd(
        nc, in_maps, core_ids=list(range(N_CORES)))
    return np.concatenate(
        [res.results[i]["out"] for i in range(N_CORES)], axis=1)

